# revision 22
# baseline (speedup 1.0000x reference)
"""DomainSpecificHeads on 8 trn2 NeuronCores.

Reference computation (per example b):
    hidden = hidden_states[b] @ W_base + b_base            # [S, D]
    out[b] = hidden @ W_heads[idx[b]] + b_heads[idx[b]]    # [S, V]
with idx = domain_ids clamped to the default head (slot ND) when out of range.

Sharding: data-parallel over batch (B == n_cores == 8). The host gathers
W_heads[idx[b]] / b_heads[idx[b]] for core b, so the MoE routing is a free
host-side slice and every core runs the identical program on its own example.

Device kernel computes out_T = [V, S] (v on partitions) so that BOTH biases
are per-partition biases, fused into the PSUM->SBUF eviction on the scalar
engine. The host transposes back when gathering.

Per-core pipeline (all matmuls bf16, 1 cycle/row on the PE):
  stage 1: hidden_T = W_base^T-contract(h) + b_base  (64 MMs, stays in SBUF)
  stage 2: stream W_heads[idx] in 2 MB chunks (Tile double-buffered,
           WH_BUFS=5), 8 accumulating K=128 MMs per [128v, 512s] PSUM tile,
           ScalarE eviction with fused bias, 2 MB output DMAs.

Measured (8-core SPMD, min over interleaved sweeps of a 48x on-device
repeat loop): ~533 us/exec, rel err 3.3e-3 vs the fp32 reference.
That is ~99% of the compute roofline: 2064 MMs x 512 cycles at the
device's sustained ~1.94 GHz PE clock (~528 us); DMA (374 us: 65.5 MB
bf16 weights in + 65.5 MB fp32 out + 3 MB stage-1) fully overlaps.
fp32/fp32r paths were rejected: plain fp32 runs at 1/4 PE rate, and
fp32r operands must be produced by a rounding instruction (BIR verifier),
which would cost an extra full-tensor pass.
"""

import numpy as np
from contextlib import ExitStack

import concourse.bass as bass
import concourse.mybir as mybir
import concourse.tile as tile
from concourse import bacc
from concourse._compat import with_exitstack
from concourse.bass_utils import run_bass_kernel_spmd

B, S, D, V, ND = 8, 512, 1024, 32000, 8
N_CORES = 8
P = 128
KO = D // P  # 8 contraction subtiles

# Tunables
W_DTYPE = "bfloat16"  # "bfloat16" | "float32r" (fp32 bytes, fp32r matmul)
V_CHUNK = 1024        # W_heads columns streamed per DMA chunk (multiple of 128)
WH_BUFS = 5
OUT_BUFS = 4
PSUM_BUFS = 8

_FP32 = mybir.dt.float32
_FP32R = mybir.dt.float32r
_BF16 = mybir.dt.bfloat16
# Stage-1 operands (h, W_base) are cast on host; bf16 is plenty for the
# 1024x1024 base projection and keeps the tensor engine at 1 cyc/row.
_DT_S1 = _BF16


BENCH_MODE = "full"  # "full" | "dma" | "pe"  (dev-only diagnosis modes)


@with_exitstack
def _kernel_body(ctx: ExitStack, tc: tile.TileContext, hT, Wb, bbT, Wh, bhT, outT,
                 dt_w):
    nc = tc.nc
    ident = mybir.ActivationFunctionType.Identity

    const = ctx.enter_context(tc.tile_pool(name="const", bufs=1))
    stage1 = ctx.enter_context(tc.tile_pool(name="stage1", bufs=1))
    whp = ctx.enter_context(tc.tile_pool(name="whp", bufs=WH_BUFS))
    outp = ctx.enter_context(tc.tile_pool(name="outp", bufs=OUT_BUFS))
    psum = ctx.enter_context(tc.tile_pool(name="psum", bufs=PSUM_BUFS, space="PSUM"))

    # ---- constants / stage-1 inputs ------------------------------------
    bb_sb = const.tile([P, KO], _FP32, tag="bb")
    nc.sync.dma_start(bb_sb[:], bbT[:])
    bh_sb = const.tile([P, V // P], _FP32, tag="bh")
    nc.sync.dma_start(bh_sb[:], bhT[:])

    hT_sb = stage1.tile([P, KO, S], _DT_S1, tag="hT")
    nc.sync.dma_start(hT_sb[:], hT.rearrange("(ko kp) s -> kp ko s", kp=P))
    Wb_sb = stage1.tile([P, KO, D], _DT_S1, tag="Wb")
    nc.sync.dma_start(Wb_sb[:], Wb.rearrange("(ko kp) e -> kp ko e", kp=P))

    # ---- stage 1: hidden_T[d', s] = sum_d W_base[d, d'] * h[s, d] + b_base
    hid_sb = stage1.tile([P, KO, S], dt_w, tag="hid")
    for m in range(KO):
        ps = psum.tile([P, S], _FP32, tag="ps")
        for k in range(KO):
            nc.tensor.matmul(
                ps[:],
                Wb_sb[:, k, m * P:(m + 1) * P],
                hT_sb[:, k, :],
                start=(k == 0),
                stop=(k == KO - 1),
            )
        nc.scalar.activation(hid_sb[:, m, :], ps[:], ident, bias=bb_sb[:, m:m + 1])

    # ---- stage 2: out_T[v, s] = sum_d' Wh[d', v] * hidden_T[d', s] + bh[v]
    whr = Wh.rearrange("(ko kp) v -> kp ko v", kp=P)
    outr = outT.rearrange("(vo vp) s -> vp vo s", vp=P)

    if BENCH_MODE == "dma":
        # Streams the real in/out traffic with no compute: wh tiles are
        # loaded and immediately recycled; one constant tile is DMA'd out.
        cst = outp.tile([P, V_CHUNK // P, S], _FP32, tag="out")
        nc.scalar.activation(cst[:, 0, :], hid_sb[:, 0, :], ident, bias=0.0)
        for j in range(1, V_CHUNK // P):
            nc.scalar.activation(cst[:, j, :], hid_sb[:, 0, :], ident, bias=0.0)
        v0 = 0
        while v0 < V:
            vc = min(V_CHUNK, V - v0)
            nj = vc // P
            wh_sb = whp.tile([P, KO, V_CHUNK], dt_w, tag="wh")
            nc.sync.dma_start(wh_sb[:, :, :vc], whr[:, :, v0:v0 + vc])
            sink = const.tile([P, 8], dt_w, tag="sink")
            nc.vector.tensor_copy(sink[:], wh_sb[:, 0, 0:8])
            nc.sync.dma_start(outr[:, v0 // P:v0 // P + nj, :], cst[:, :nj, :])
            v0 += vc
        return
    if BENCH_MODE in ("pe", "pe256"):
        # Full matmul/eviction stream on a single resident wh chunk.
        half = BENCH_MODE == "pe256"
        wh_sb = whp.tile([P, KO, V_CHUNK], dt_w, tag="wh")
        nc.sync.dma_start(wh_sb[:], whr[:, :, 0:V_CHUNK])
        v0 = 0
        while v0 < V:
            vc = min(V_CHUNK, V - v0)
            nj = vc // P
            out_sb = outp.tile([P, V_CHUNK // P, S], _FP32, tag="out")
            for j in range(nj):
                ps = psum.tile([P, S], _FP32, tag="ps")
                for k in range(KO):
                    if half:
                        nc.tensor.matmul(
                            ps[:, 0:S // 2],
                            wh_sb[:, k, j * P:(j + 1) * P],
                            hid_sb[:, k, 0:S // 2],
                            start=(k == 0), stop=(k == KO - 1),
                        )
                        nc.tensor.matmul(
                            ps[:, S // 2:S],
                            wh_sb[:, k, j * P:(j + 1) * P],
                            hid_sb[:, k, S // 2:S],
                            start=(k == 0), stop=(k == KO - 1),
                        )
                    else:
                        nc.tensor.matmul(
                            ps[:],
                            wh_sb[:, k, j * P:(j + 1) * P],
                            hid_sb[:, k, :],
                            start=(k == 0),
                            stop=(k == KO - 1),
                        )
                t = v0 // P + j
                nc.scalar.activation(out_sb[:, j, :], ps[:], ident,
                                     bias=bh_sb[:, t:t + 1])
            v0 += vc
        nc.sync.dma_start(outr[:, 0:V_CHUNK // P, :], out_sb[:])
        return

    v0 = 0
    while v0 < V:
        vc = min(V_CHUNK, V - v0)
        nj = vc // P
        wh_sb = whp.tile([P, KO, V_CHUNK], dt_w, tag="wh")
        nc.sync.dma_start(wh_sb[:, :, :vc], whr[:, :, v0:v0 + vc])
        out_sb = outp.tile([P, V_CHUNK // P, S], _FP32, tag="out")
        for j in range(nj):
            ps = psum.tile([P, S], _FP32, tag="ps")
            for k in range(KO):
                nc.tensor.matmul(
                    ps[:],
                    wh_sb[:, k, j * P:(j + 1) * P],
                    hid_sb[:, k, :],
                    start=(k == 0),
                    stop=(k == KO - 1),
                )
            t = v0 // P + j
            nc.scalar.activation(out_sb[:, j, :], ps[:], ident, bias=bh_sb[:, t:t + 1])
        nc.sync.dma_start(outr[:, v0 // P:v0 // P + nj, :], out_sb[:, :nj, :])
        v0 += vc


_NC_CACHE = {}


def _build_nc(dt_w, n_reps=1):
    key = (str(dt_w), n_reps, BENCH_MODE)
    if key in _NC_CACHE:
        return _NC_CACHE[key]
    nc = bacc.Bacc("TRN2", target_bir_lowering=False, debug=False,
                   num_devices=N_CORES)
    hT = nc.dram_tensor("hT", [D, S], _DT_S1, kind="ExternalInput").ap()
    Wb = nc.dram_tensor("Wb", [D, D], _DT_S1, kind="ExternalInput").ap()
    bbT = nc.dram_tensor("bbT", [P, KO], _FP32, kind="ExternalInput").ap()
    Wh = nc.dram_tensor("Wh", [D, V], dt_w, kind="ExternalInput").ap()
    bhT = nc.dram_tensor("bhT", [P, V // P], _FP32, kind="ExternalInput").ap()
    outT = nc.dram_tensor("outT", [V, S], _FP32, kind="ExternalOutput").ap()
    with tile.TileContext(nc) as tc:
        if n_reps == 1:
            _kernel_body(tc, hT, Wb, bbT, Wh, bhT, outT, dt_w)
        else:
            # Bench-only: repeat the whole computation on-device so the
            # per-iteration time can be separated from dispatch overhead.
            with tc.For_i(0, n_reps, 1):
                _kernel_body(tc, hT, Wb, bbT, Wh, bhT, outT, dt_w)
    nc.compile()
    _NC_CACHE[key] = nc
    return nc


def _make_in_maps(hidden_states, domain_ids, W_base, b_base, W_heads, b_heads,
                  dt_w):
    hidden_states = np.asarray(hidden_states, dtype=np.float32)
    domain_ids = np.asarray(domain_ids)
    W_base = np.ascontiguousarray(np.asarray(W_base, dtype=np.float32))
    b_base = np.asarray(b_base, dtype=np.float32)
    W_heads = np.asarray(W_heads, dtype=np.float32)
    b_heads = np.asarray(b_heads, dtype=np.float32)

    nd = W_heads.shape[0] - 1
    ids = domain_ids.astype(np.int64)
    idx = np.where((ids >= 0) & (ids < nd), ids, nd)

    import ml_dtypes
    bf16 = ml_dtypes.bfloat16
    bbT = np.ascontiguousarray(b_base.reshape(KO, P).T)
    np_w = bf16 if dt_w == _BF16 else np.float32
    Wb_s1 = np.ascontiguousarray(W_base.astype(bf16))

    wh_cache, bh_cache = {}, {}
    in_maps = []
    for b in range(B):
        i = int(idx[b])
        if i not in wh_cache:
            wh_cache[i] = np.ascontiguousarray(
                W_heads[i].astype(np_w, copy=False))
            bh_cache[i] = np.ascontiguousarray(
                b_heads[i].reshape(V // P, P).T)
        in_maps.append({
            "hT": np.ascontiguousarray(hidden_states[b].T.astype(bf16)),
            "Wb": Wb_s1,
            "bbT": bbT,
            "Wh": wh_cache[i],
            "bhT": bh_cache[i],
        })
    return in_maps


def _gather_out(results):
    out = np.empty((B, S, V), dtype=np.float32)
    for b in range(B):
        out[b] = results[b]["outT"].T
    return out


def run_raw(trace=False, **inputs):
    """Run on hardware; returns (out [B,S,V] fp32, BassKernelResults)."""
    dt_w = _BF16 if W_DTYPE == "bfloat16" else _FP32R
    nc = _build_nc(dt_w)
    in_maps = _make_in_maps(
        inputs["hidden_states"], inputs["domain_ids"], inputs["W_base"],
        inputs["b_base"], inputs["W_heads"], inputs["b_heads"], dt_w)
    res = run_bass_kernel_spmd(nc, in_maps, core_ids=list(range(N_CORES)),
                               trace=trace)
    return _gather_out(res.results), res


def kernel(**inputs) -> np.ndarray:
    out, _ = run_raw(trace=False, **inputs)
    return out


# ---------------------------------------------------------------------------
# Dev-only helpers below (not used by kernel()).
# ---------------------------------------------------------------------------

def predict_ns():
    """Cost-model (TimelineSim) predicted single-core duration in ns."""
    from concourse.timeline_sim import TimelineSim
    dt_w = _BF16 if W_DTYPE == "bfloat16" else _FP32R
    nc = _build_nc(dt_w)
    tl = TimelineSim(nc, trace=False)
    return tl.simulate()


def _make_runner(nc, in_maps):
    """Build a jitted single-dispatch runner over device-resident inputs.
    Returns (run_once, to_out_maps)."""
    import jax
    from jax.sharding import Mesh, PartitionSpec, NamedSharding
    from jax.experimental.shard_map import shard_map
    from concourse import bass2jax
    from concourse import mybir as _mybir

    bass2jax.install_neuronx_cc_hook()
    partition_name = (nc.partition_id_tensor.name
                      if nc.partition_id_tensor else None)
    in_names, out_names, out_avals, zero_outs = [], [], [], []
    for alloc in nc.m.functions[0].allocations:
        if not isinstance(alloc, _mybir.MemoryLocationSet):
            continue
        name = alloc.memorylocations[0].name
        if alloc.kind == "ExternalInput":
            if name != partition_name:
                in_names.append(name)
        elif alloc.kind == "ExternalOutput":
            out_names.append(name)
            shape = tuple(alloc.tensor_shape)
            dtype = _mybir.dt.np(alloc.dtype)
            out_avals.append(jax.core.ShapedArray(shape, dtype))
            zero_outs.append(np.zeros(shape, dtype))
    n_params = len(in_names)
    n_outs = len(out_avals)
    all_names = in_names + out_names
    if partition_name is not None:
        all_names = all_names + [partition_name]

    def _body(*args):
        operands = list(args)
        if partition_name is not None:
            operands.append(bass2jax.partition_id_tensor())
        return tuple(bass2jax._bass_exec_p.bind(
            *operands,
            out_avals=tuple(out_avals),
            in_names=tuple(all_names),
            out_names=tuple(out_names),
            lowering_input_output_aliases=(),
            sim_require_finite=True,
            sim_require_nnan=True,
            nc=nc,
        ))

    devices = jax.devices()[:N_CORES]
    mesh = Mesh(np.asarray(devices), ("core",))
    spec = PartitionSpec("core")
    f = jax.jit(
        shard_map(_body, mesh=mesh, in_specs=(spec,) * (n_params + n_outs),
                  out_specs=(spec,) * n_outs, check_rep=False),
        keep_unused=True)

    sharding = NamedSharding(mesh, spec)
    concat_in = [
        jax.device_put(
            np.concatenate([np.asarray(in_maps[c][nm]) for c in range(N_CORES)],
                           axis=0), sharding)
        for nm in in_names
    ]
    jax.block_until_ready(concat_in)

    z = [jax.device_put(
        np.zeros((N_CORES * zz.shape[0], *zz.shape[1:]), zz.dtype), sharding)
        for zz in zero_outs]
    jax.block_until_ready(z)

    def run_once():
        import time
        t0 = time.perf_counter()
        outs = f(*concat_in, *z)
        jax.block_until_ready(outs)
        return time.perf_counter() - t0, outs

    def to_out_maps(outs):
        return [
            {nm: np.asarray(outs[i]).reshape(N_CORES, *out_avals[i].shape)[c]
             for i, nm in enumerate(out_names)}
            for c in range(N_CORES)
        ]

    return run_once, to_out_maps


def bench(n_iters=16, **inputs):
    """Measure per-kernel HW time: build a NEFF that repeats the body
    n_iters times in a hardware loop and difference against the 1-rep NEFF.
    Returns (out, per_iter_ns, first_total_ns)."""
    import time
    dt_w = _BF16 if W_DTYPE == "bfloat16" else _FP32R
    in_maps = _make_in_maps(
        inputs["hidden_states"], inputs["domain_ids"], inputs["W_base"],
        inputs["b_base"], inputs["W_heads"], inputs["b_heads"], dt_w)

    nc1 = _build_nc(dt_w, 1)
    run1, to_out_maps = _make_runner(nc1, in_maps)
    t0 = time.perf_counter()
    _, outs = run1()
    first_total = time.perf_counter() - t0

    ncn = _build_nc(dt_w, n_iters)
    runn, _ = _make_runner(ncn, in_maps)
    runn()  # warm

    # Interleaved sweeps: cross-run drift on the shared device is large, so
    # take the min per-iteration estimate across several paired measurements.
    per_iter = float("inf")
    for _ in range(4):
        t1 = min(run1()[0] for _ in range(3))
        tn = min(runn()[0] for _ in range(3))
        per_iter = min(per_iter, (tn - t1) / (n_iters - 1))
    _, outs = run1()

    return _gather_out(to_out_maps(outs)), per_iter * 1e9, first_total * 1e9


# revision 23
# speedup vs baseline: 1.0054x; 1.0054x over previous
"""DomainSpecificHeads on 8 trn2 NeuronCores.

Reference computation (per example b):
    hidden = hidden_states[b] @ W_base + b_base            # [S, D]
    out[b] = hidden @ W_heads[idx[b]] + b_heads[idx[b]]    # [S, V]
with idx = domain_ids clamped to the default head (slot ND) when out of range.

Sharding: data-parallel over batch (B == n_cores == 8). The host gathers
W_heads[idx[b]] / b_heads[idx[b]] for core b, so the MoE routing is a free
host-side slice and every core runs the identical program on its own example.

Device kernel computes out_T = [V, S] (v on partitions) so that BOTH biases
are per-partition biases, fused into the PSUM->SBUF eviction on the scalar
engine. The host transposes back when gathering.

Per-core pipeline (all matmuls bf16, 1 cycle/row on the PE):
  stage 1: hidden_T = W_base^T-contract(h) + b_base  (64 MMs, stays in SBUF)
  stage 2: stream W_heads[idx] in 2 MB chunks (Tile double-buffered,
           WH_BUFS=5), 8 accumulating K=128 MMs per [128v, 512s] PSUM tile,
           ScalarE eviction with fused bias, 2 MB output DMAs.

Measured (8-core SPMD, min over interleaved sweeps of a 48x on-device
repeat loop): ~533 us/exec, rel err 3.3e-3 vs the fp32 reference.
That is ~99% of the compute roofline: 2064 MMs x 512 cycles at the
device's sustained ~1.94 GHz PE clock (~528 us); DMA (374 us: 65.5 MB
bf16 weights in + 65.5 MB fp32 out + 3 MB stage-1) fully overlaps.
fp32/fp32r paths were rejected: plain fp32 runs at 1/4 PE rate, and
fp32r operands must be produced by a rounding instruction (BIR verifier),
which would cost an extra full-tensor pass.
"""

import numpy as np
from contextlib import ExitStack

import concourse.bass as bass
import concourse.mybir as mybir
import concourse.tile as tile
from concourse import bacc
from concourse._compat import with_exitstack
from concourse.bass_utils import run_bass_kernel_spmd

B, S, D, V, ND = 8, 512, 1024, 32000, 8
N_CORES = 8
P = 128
KO = D // P  # 8 contraction subtiles

# Tunables
W_DTYPE = "bfloat16"  # "bfloat16" | "float32r" (fp32 bytes, fp32r matmul)
V_CHUNK = 1024        # W_heads columns streamed per DMA chunk (multiple of 128)
WH_BUFS = 5
OUT_BUFS = 4
PSUM_BUFS = 8

_FP32 = mybir.dt.float32
_FP32R = mybir.dt.float32r
_BF16 = mybir.dt.bfloat16
# Stage-1 operands (h, W_base) are cast on host; bf16 is plenty for the
# 1024x1024 base projection and keeps the tensor engine at 1 cyc/row.
_DT_S1 = _BF16


BENCH_MODE = "full"  # "full" | "dma" | "pe"  (dev-only diagnosis modes)
EVICT = "act"        # "act" | "dve" | "mix" — engine for PSUM->SBUF eviction


@with_exitstack
def _kernel_body(ctx: ExitStack, tc: tile.TileContext, hT, Wb, bbT, Wh, bhT, outT,
                 dt_w):
    nc = tc.nc
    ident = mybir.ActivationFunctionType.Identity

    def evict(dst, ps, bias_col, j):
        eng = EVICT if EVICT != "mix" else ("act" if j % 2 == 0 else "dve")
        if eng == "act":
            nc.scalar.activation(dst, ps, ident, bias=bias_col)
        else:
            nc.vector.tensor_scalar_add(dst, ps, bias_col)

    const = ctx.enter_context(tc.tile_pool(name="const", bufs=1))
    stage1 = ctx.enter_context(tc.tile_pool(name="stage1", bufs=1))
    whp = ctx.enter_context(tc.tile_pool(name="whp", bufs=WH_BUFS))
    outp = ctx.enter_context(tc.tile_pool(name="outp", bufs=OUT_BUFS))
    psum = ctx.enter_context(tc.tile_pool(name="psum", bufs=PSUM_BUFS, space="PSUM"))

    # ---- constants / stage-1 inputs ------------------------------------
    bb_sb = const.tile([P, KO], _FP32, tag="bb")
    nc.sync.dma_start(bb_sb[:], bbT[:])
    bh_sb = const.tile([P, V // P], _FP32, tag="bh")
    nc.sync.dma_start(bh_sb[:], bhT[:])

    hT_sb = stage1.tile([P, KO, S], _DT_S1, tag="hT")
    nc.sync.dma_start(hT_sb[:], hT.rearrange("(ko kp) s -> kp ko s", kp=P))
    Wb_sb = stage1.tile([P, KO, D], _DT_S1, tag="Wb")
    nc.sync.dma_start(Wb_sb[:], Wb.rearrange("(ko kp) e -> kp ko e", kp=P))

    # ---- stage 1: hidden_T[d', s] = sum_d W_base[d, d'] * h[s, d] + b_base
    hid_sb = stage1.tile([P, KO, S], dt_w, tag="hid")
    for m in range(KO):
        ps = psum.tile([P, S], _FP32, tag="ps")
        for k in range(KO):
            nc.tensor.matmul(
                ps[:],
                Wb_sb[:, k, m * P:(m + 1) * P],
                hT_sb[:, k, :],
                start=(k == 0),
                stop=(k == KO - 1),
            )
        evict(hid_sb[:, m, :], ps[:], bb_sb[:, m:m + 1], m)

    # ---- stage 2: out_T[v, s] = sum_d' Wh[d', v] * hidden_T[d', s] + bh[v]
    whr = Wh.rearrange("(ko kp) v -> kp ko v", kp=P)
    outr = outT.rearrange("(vo vp) s -> vp vo s", vp=P)

    if BENCH_MODE == "dma":
        # Streams the real in/out traffic with no compute: wh tiles are
        # loaded and immediately recycled; one constant tile is DMA'd out.
        cst = outp.tile([P, V_CHUNK // P, S], _FP32, tag="out")
        nc.scalar.activation(cst[:, 0, :], hid_sb[:, 0, :], ident, bias=0.0)
        for j in range(1, V_CHUNK // P):
            nc.scalar.activation(cst[:, j, :], hid_sb[:, 0, :], ident, bias=0.0)
        v0 = 0
        while v0 < V:
            vc = min(V_CHUNK, V - v0)
            nj = vc // P
            wh_sb = whp.tile([P, KO, V_CHUNK], dt_w, tag="wh")
            nc.sync.dma_start(wh_sb[:, :, :vc], whr[:, :, v0:v0 + vc])
            sink = const.tile([P, 8], dt_w, tag="sink")
            nc.vector.tensor_copy(sink[:], wh_sb[:, 0, 0:8])
            nc.sync.dma_start(outr[:, v0 // P:v0 // P + nj, :], cst[:, :nj, :])
            v0 += vc
        return
    if BENCH_MODE in ("pe", "pe256"):
        # Full matmul/eviction stream on a single resident wh chunk.
        half = BENCH_MODE == "pe256"
        wh_sb = whp.tile([P, KO, V_CHUNK], dt_w, tag="wh")
        nc.sync.dma_start(wh_sb[:], whr[:, :, 0:V_CHUNK])
        v0 = 0
        while v0 < V:
            vc = min(V_CHUNK, V - v0)
            nj = vc // P
            out_sb = outp.tile([P, V_CHUNK // P, S], _FP32, tag="out")
            for j in range(nj):
                ps = psum.tile([P, S], _FP32, tag="ps")
                for k in range(KO):
                    if half:
                        nc.tensor.matmul(
                            ps[:, 0:S // 2],
                            wh_sb[:, k, j * P:(j + 1) * P],
                            hid_sb[:, k, 0:S // 2],
                            start=(k == 0), stop=(k == KO - 1),
                        )
                        nc.tensor.matmul(
                            ps[:, S // 2:S],
                            wh_sb[:, k, j * P:(j + 1) * P],
                            hid_sb[:, k, S // 2:S],
                            start=(k == 0), stop=(k == KO - 1),
                        )
                    else:
                        nc.tensor.matmul(
                            ps[:],
                            wh_sb[:, k, j * P:(j + 1) * P],
                            hid_sb[:, k, :],
                            start=(k == 0),
                            stop=(k == KO - 1),
                        )
                t = v0 // P + j
                evict(out_sb[:, j, :], ps[:], bh_sb[:, t:t + 1], t)
            v0 += vc
        nc.sync.dma_start(outr[:, 0:V_CHUNK // P, :], out_sb[:])
        return

    v0 = 0
    while v0 < V:
        vc = min(V_CHUNK, V - v0)
        nj = vc // P
        wh_sb = whp.tile([P, KO, V_CHUNK], dt_w, tag="wh")
        nc.sync.dma_start(wh_sb[:, :, :vc], whr[:, :, v0:v0 + vc])
        out_sb = outp.tile([P, V_CHUNK // P, S], _FP32, tag="out")
        for j in range(nj):
            ps = psum.tile([P, S], _FP32, tag="ps")
            for k in range(KO):
                nc.tensor.matmul(
                    ps[:],
                    wh_sb[:, k, j * P:(j + 1) * P],
                    hid_sb[:, k, :],
                    start=(k == 0),
                    stop=(k == KO - 1),
                )
            t = v0 // P + j
            evict(out_sb[:, j, :], ps[:], bh_sb[:, t:t + 1], t)
        nc.sync.dma_start(outr[:, v0 // P:v0 // P + nj, :], out_sb[:, :nj, :])
        v0 += vc


_NC_CACHE = {}


def _build_nc(dt_w, n_reps=1):
    key = (str(dt_w), n_reps, BENCH_MODE, EVICT)
    if key in _NC_CACHE:
        return _NC_CACHE[key]
    nc = bacc.Bacc("TRN2", target_bir_lowering=False, debug=False,
                   num_devices=N_CORES)
    hT = nc.dram_tensor("hT", [D, S], _DT_S1, kind="ExternalInput").ap()
    Wb = nc.dram_tensor("Wb", [D, D], _DT_S1, kind="ExternalInput").ap()
    bbT = nc.dram_tensor("bbT", [P, KO], _FP32, kind="ExternalInput").ap()
    Wh = nc.dram_tensor("Wh", [D, V], dt_w, kind="ExternalInput").ap()
    bhT = nc.dram_tensor("bhT", [P, V // P], _FP32, kind="ExternalInput").ap()
    outT = nc.dram_tensor("outT", [V, S], _FP32, kind="ExternalOutput").ap()
    with tile.TileContext(nc) as tc:
        if n_reps == 1:
            _kernel_body(tc, hT, Wb, bbT, Wh, bhT, outT, dt_w)
        else:
            # Bench-only: repeat the whole computation on-device so the
            # per-iteration time can be separated from dispatch overhead.
            with tc.For_i(0, n_reps, 1):
                _kernel_body(tc, hT, Wb, bbT, Wh, bhT, outT, dt_w)
    nc.compile()
    _NC_CACHE[key] = nc
    return nc


def _make_in_maps(hidden_states, domain_ids, W_base, b_base, W_heads, b_heads,
                  dt_w):
    hidden_states = np.asarray(hidden_states, dtype=np.float32)
    domain_ids = np.asarray(domain_ids)
    W_base = np.ascontiguousarray(np.asarray(W_base, dtype=np.float32))
    b_base = np.asarray(b_base, dtype=np.float32)
    W_heads = np.asarray(W_heads, dtype=np.float32)
    b_heads = np.asarray(b_heads, dtype=np.float32)

    nd = W_heads.shape[0] - 1
    ids = domain_ids.astype(np.int64)
    idx = np.where((ids >= 0) & (ids < nd), ids, nd)

    import ml_dtypes
    bf16 = ml_dtypes.bfloat16
    bbT = np.ascontiguousarray(b_base.reshape(KO, P).T)
    np_w = bf16 if dt_w == _BF16 else np.float32
    Wb_s1 = np.ascontiguousarray(W_base.astype(bf16))

    wh_cache, bh_cache = {}, {}
    in_maps = []
    for b in range(B):
        i = int(idx[b])
        if i not in wh_cache:
            wh_cache[i] = np.ascontiguousarray(
                W_heads[i].astype(np_w, copy=False))
            bh_cache[i] = np.ascontiguousarray(
                b_heads[i].reshape(V // P, P).T)
        in_maps.append({
            "hT": np.ascontiguousarray(hidden_states[b].T.astype(bf16)),
            "Wb": Wb_s1,
            "bbT": bbT,
            "Wh": wh_cache[i],
            "bhT": bh_cache[i],
        })
    return in_maps


def _gather_out(results):
    out = np.empty((B, S, V), dtype=np.float32)
    for b in range(B):
        out[b] = results[b]["outT"].T
    return out


def run_raw(trace=False, **inputs):
    """Run on hardware; returns (out [B,S,V] fp32, BassKernelResults)."""
    dt_w = _BF16 if W_DTYPE == "bfloat16" else _FP32R
    nc = _build_nc(dt_w)
    in_maps = _make_in_maps(
        inputs["hidden_states"], inputs["domain_ids"], inputs["W_base"],
        inputs["b_base"], inputs["W_heads"], inputs["b_heads"], dt_w)
    res = run_bass_kernel_spmd(nc, in_maps, core_ids=list(range(N_CORES)),
                               trace=trace)
    return _gather_out(res.results), res


def kernel(**inputs) -> np.ndarray:
    out, _ = run_raw(trace=False, **inputs)
    return out


# ---------------------------------------------------------------------------
# Dev-only helpers below (not used by kernel()).
# ---------------------------------------------------------------------------

def predict_ns():
    """Cost-model (TimelineSim) predicted single-core duration in ns."""
    from concourse.timeline_sim import TimelineSim
    dt_w = _BF16 if W_DTYPE == "bfloat16" else _FP32R
    nc = _build_nc(dt_w)
    tl = TimelineSim(nc, trace=False)
    return tl.simulate()


def _make_runner(nc, in_maps):
    """Build a jitted single-dispatch runner over device-resident inputs.
    Returns (run_once, to_out_maps)."""
    import jax
    from jax.sharding import Mesh, PartitionSpec, NamedSharding
    from jax.experimental.shard_map import shard_map
    from concourse import bass2jax
    from concourse import mybir as _mybir

    bass2jax.install_neuronx_cc_hook()
    partition_name = (nc.partition_id_tensor.name
                      if nc.partition_id_tensor else None)
    in_names, out_names, out_avals, zero_outs = [], [], [], []
    for alloc in nc.m.functions[0].allocations:
        if not isinstance(alloc, _mybir.MemoryLocationSet):
            continue
        name = alloc.memorylocations[0].name
        if alloc.kind == "ExternalInput":
            if name != partition_name:
                in_names.append(name)
        elif alloc.kind == "ExternalOutput":
            out_names.append(name)
            shape = tuple(alloc.tensor_shape)
            dtype = _mybir.dt.np(alloc.dtype)
            out_avals.append(jax.core.ShapedArray(shape, dtype))
            zero_outs.append(np.zeros(shape, dtype))
    n_params = len(in_names)
    n_outs = len(out_avals)
    all_names = in_names + out_names
    if partition_name is not None:
        all_names = all_names + [partition_name]

    def _body(*args):
        operands = list(args)
        if partition_name is not None:
            operands.append(bass2jax.partition_id_tensor())
        return tuple(bass2jax._bass_exec_p.bind(
            *operands,
            out_avals=tuple(out_avals),
            in_names=tuple(all_names),
            out_names=tuple(out_names),
            lowering_input_output_aliases=(),
            sim_require_finite=True,
            sim_require_nnan=True,
            nc=nc,
        ))

    devices = jax.devices()[:N_CORES]
    mesh = Mesh(np.asarray(devices), ("core",))
    spec = PartitionSpec("core")
    f = jax.jit(
        shard_map(_body, mesh=mesh, in_specs=(spec,) * (n_params + n_outs),
                  out_specs=(spec,) * n_outs, check_rep=False),
        keep_unused=True)

    sharding = NamedSharding(mesh, spec)
    concat_in = [
        jax.device_put(
            np.concatenate([np.asarray(in_maps[c][nm]) for c in range(N_CORES)],
                           axis=0), sharding)
        for nm in in_names
    ]
    jax.block_until_ready(concat_in)

    z = [jax.device_put(
        np.zeros((N_CORES * zz.shape[0], *zz.shape[1:]), zz.dtype), sharding)
        for zz in zero_outs]
    jax.block_until_ready(z)

    def run_once():
        import time
        t0 = time.perf_counter()
        outs = f(*concat_in, *z)
        jax.block_until_ready(outs)
        return time.perf_counter() - t0, outs

    def to_out_maps(outs):
        return [
            {nm: np.asarray(outs[i]).reshape(N_CORES, *out_avals[i].shape)[c]
             for i, nm in enumerate(out_names)}
            for c in range(N_CORES)
        ]

    return run_once, to_out_maps


def bench(n_iters=16, **inputs):
    """Measure per-kernel HW time: build a NEFF that repeats the body
    n_iters times in a hardware loop and difference against the 1-rep NEFF.
    Returns (out, per_iter_ns, first_total_ns)."""
    import time
    dt_w = _BF16 if W_DTYPE == "bfloat16" else _FP32R
    in_maps = _make_in_maps(
        inputs["hidden_states"], inputs["domain_ids"], inputs["W_base"],
        inputs["b_base"], inputs["W_heads"], inputs["b_heads"], dt_w)

    nc1 = _build_nc(dt_w, 1)
    run1, to_out_maps = _make_runner(nc1, in_maps)
    t0 = time.perf_counter()
    _, outs = run1()
    first_total = time.perf_counter() - t0

    ncn = _build_nc(dt_w, n_iters)
    runn, _ = _make_runner(ncn, in_maps)
    runn()  # warm

    # Interleaved sweeps: cross-run drift on the shared device is large, so
    # take the min per-iteration estimate across several paired measurements.
    per_iter = float("inf")
    for _ in range(4):
        t1 = min(run1()[0] for _ in range(3))
        tn = min(runn()[0] for _ in range(3))
        per_iter = min(per_iter, (tn - t1) / (n_iters - 1))
    _, outs = run1()

    return _gather_out(to_out_maps(outs)), per_iter * 1e9, first_total * 1e9


# revision 25
# speedup vs baseline: 1.0410x; 1.0354x over previous
"""DomainSpecificHeads on 8 trn2 NeuronCores.

Reference computation (per example b):
    hidden = hidden_states[b] @ W_base + b_base            # [S, D]
    out[b] = hidden @ W_heads[idx[b]] + b_heads[idx[b]]    # [S, V]
with idx = domain_ids clamped to the default head (slot ND) when out of range.

Sharding: data-parallel over batch (B == n_cores == 8). The host gathers
W_heads[idx[b]] / b_heads[idx[b]] for core b, so the MoE routing is a free
host-side slice and every core runs the identical program on its own example.

Device kernel computes out_T = [V, S] (v on partitions) so that BOTH biases
are per-partition biases, fused into the PSUM->SBUF eviction on the scalar
engine. The host transposes back when gathering.

Per-core pipeline (all matmuls bf16, 1 cycle/row on the PE):
  stage 1: hidden_T = W_base^T-contract(h) + b_base  (64 MMs, stays in SBUF)
  stage 2: stream W_heads[idx] in 2 MB chunks (Tile double-buffered,
           WH_BUFS=5), 16 accumulating K=128 MMs per paired 2-bank PSUM
           tile (halves group-boundary waits on the PE sequencer),
           ScalarE eviction with fused bias, 2 MB output DMAs.

Measured (8-core SPMD, min over interleaved sweeps of a 48x on-device
repeat loop): ~533 us/exec, rel err 3.3e-3 vs the fp32 reference.
That is ~99% of the compute roofline: 2064 MMs x 512 cycles at the
device's sustained ~1.94 GHz PE clock (~528 us); DMA (374 us: 65.5 MB
bf16 weights in + 65.5 MB fp32 out + 3 MB stage-1) fully overlaps.
fp32/fp32r paths were rejected: plain fp32 runs at 1/4 PE rate, and
fp32r operands must be produced by a rounding instruction (BIR verifier),
which would cost an extra full-tensor pass.
"""

import numpy as np
from contextlib import ExitStack

import concourse.bass as bass
import concourse.mybir as mybir
import concourse.tile as tile
from concourse import bacc
from concourse._compat import with_exitstack
from concourse.bass_utils import run_bass_kernel_spmd

B, S, D, V, ND = 8, 512, 1024, 32000, 8
N_CORES = 8
P = 128
KO = D // P  # 8 contraction subtiles

# Tunables
W_DTYPE = "bfloat16"  # "bfloat16" | "float32r" (fp32 bytes, fp32r matmul)
V_CHUNK = 1024        # W_heads columns streamed per DMA chunk (multiple of 128)
WH_BUFS = 5
OUT_BUFS = 4
PSUM_BUFS = 8

_FP32 = mybir.dt.float32
_FP32R = mybir.dt.float32r
_BF16 = mybir.dt.bfloat16
# Stage-1 operands (h, W_base) are cast on host; bf16 is plenty for the
# 1024x1024 base projection and keeps the tensor engine at 1 cyc/row.
_DT_S1 = _BF16


BENCH_MODE = "full"  # "full" | "dma" | "pe"  (dev-only diagnosis modes)
EVICT = "act"        # "act" | "dve" | "mix" — engine for PSUM->SBUF eviction
PAIR_PSUM = True     # pair j-subtiles into 2-bank PSUM groups (16-MM groups)


@with_exitstack
def _kernel_body(ctx: ExitStack, tc: tile.TileContext, hT, Wb, bbT, Wh, bhT, outT,
                 dt_w):
    nc = tc.nc
    ident = mybir.ActivationFunctionType.Identity

    def evict(dst, ps, bias_col, j):
        eng = EVICT if EVICT != "mix" else ("act" if j % 2 == 0 else "dve")
        if eng == "act":
            nc.scalar.activation(dst, ps, ident, bias=bias_col)
        else:
            nc.vector.tensor_scalar_add(dst, ps, bias_col)

    const = ctx.enter_context(tc.tile_pool(name="const", bufs=1))
    stage1 = ctx.enter_context(tc.tile_pool(name="stage1", bufs=1))
    whp = ctx.enter_context(tc.tile_pool(name="whp", bufs=WH_BUFS))
    outp = ctx.enter_context(tc.tile_pool(name="outp", bufs=OUT_BUFS))
    ps_bufs = 2 if PAIR_PSUM else PSUM_BUFS
    psum = ctx.enter_context(tc.tile_pool(name="psum", bufs=ps_bufs, space="PSUM"))

    # ---- constants / stage-1 inputs ------------------------------------
    bb_sb = const.tile([P, KO], _FP32, tag="bb")
    nc.sync.dma_start(bb_sb[:], bbT[:])
    bh_sb = const.tile([P, V // P], _FP32, tag="bh")
    nc.sync.dma_start(bh_sb[:], bhT[:])

    hT_sb = stage1.tile([P, KO, S], _DT_S1, tag="hT")
    nc.sync.dma_start(hT_sb[:], hT.rearrange("(ko kp) s -> kp ko s", kp=P))
    Wb_sb = stage1.tile([P, KO, D], _DT_S1, tag="Wb")
    nc.sync.dma_start(Wb_sb[:], Wb.rearrange("(ko kp) e -> kp ko e", kp=P))

    # ---- stage 1: hidden_T[d', s] = sum_d W_base[d, d'] * h[s, d] + b_base
    hid_sb = stage1.tile([P, KO, S], dt_w, tag="hid")
    for m in range(KO):
        ps = psum.tile([P, S], _FP32, tag="ps")
        for k in range(KO):
            nc.tensor.matmul(
                ps[:],
                Wb_sb[:, k, m * P:(m + 1) * P],
                hT_sb[:, k, :],
                start=(k == 0),
                stop=(k == KO - 1),
            )
        evict(hid_sb[:, m, :], ps[:], bb_sb[:, m:m + 1], m)

    # ---- stage 2: out_T[v, s] = sum_d' Wh[d', v] * hidden_T[d', s] + bh[v]
    whr = Wh.rearrange("(ko kp) v -> kp ko v", kp=P)
    outr = outT.rearrange("(vo vp) s -> vp vo s", vp=P)

    if BENCH_MODE == "dma":
        # Streams the real in/out traffic with no compute: wh tiles are
        # loaded and immediately recycled; one constant tile is DMA'd out.
        cst = outp.tile([P, V_CHUNK // P, S], _FP32, tag="out")
        nc.scalar.activation(cst[:, 0, :], hid_sb[:, 0, :], ident, bias=0.0)
        for j in range(1, V_CHUNK // P):
            nc.scalar.activation(cst[:, j, :], hid_sb[:, 0, :], ident, bias=0.0)
        v0 = 0
        while v0 < V:
            vc = min(V_CHUNK, V - v0)
            nj = vc // P
            wh_sb = whp.tile([P, KO, V_CHUNK], dt_w, tag="wh")
            nc.sync.dma_start(wh_sb[:, :, :vc], whr[:, :, v0:v0 + vc])
            sink = const.tile([P, 8], dt_w, tag="sink")
            nc.vector.tensor_copy(sink[:], wh_sb[:, 0, 0:8])
            nc.sync.dma_start(outr[:, v0 // P:v0 // P + nj, :], cst[:, :nj, :])
            v0 += vc
        return
    if BENCH_MODE in ("pe", "pe256"):
        # Full matmul/eviction stream on a single resident wh chunk.
        half = BENCH_MODE == "pe256"
        wh_sb = whp.tile([P, KO, V_CHUNK], dt_w, tag="wh")
        nc.sync.dma_start(wh_sb[:], whr[:, :, 0:V_CHUNK])
        v0 = 0
        while v0 < V:
            vc = min(V_CHUNK, V - v0)
            nj = vc // P
            out_sb = outp.tile([P, V_CHUNK // P, S], _FP32, tag="out")
            for j in range(nj):
                ps = psum.tile([P, S], _FP32, tag="ps")
                for k in range(KO):
                    if half:
                        nc.tensor.matmul(
                            ps[:, 0:S // 2],
                            wh_sb[:, k, j * P:(j + 1) * P],
                            hid_sb[:, k, 0:S // 2],
                            start=(k == 0), stop=(k == KO - 1),
                        )
                        nc.tensor.matmul(
                            ps[:, S // 2:S],
                            wh_sb[:, k, j * P:(j + 1) * P],
                            hid_sb[:, k, S // 2:S],
                            start=(k == 0), stop=(k == KO - 1),
                        )
                    else:
                        nc.tensor.matmul(
                            ps[:],
                            wh_sb[:, k, j * P:(j + 1) * P],
                            hid_sb[:, k, :],
                            start=(k == 0),
                            stop=(k == KO - 1),
                        )
                t = v0 // P + j
                evict(out_sb[:, j, :], ps[:], bh_sb[:, t:t + 1], t)
            v0 += vc
        nc.sync.dma_start(outr[:, 0:V_CHUNK // P, :], out_sb[:])
        return

    v0 = 0
    while v0 < V:
        vc = min(V_CHUNK, V - v0)
        nj = vc // P
        wh_sb = whp.tile([P, KO, V_CHUNK], dt_w, tag="wh")
        nc.sync.dma_start(wh_sb[:, :, :vc], whr[:, :, v0:v0 + vc])
        out_sb = outp.tile([P, V_CHUNK // P, S], _FP32, tag="out")
        if PAIR_PSUM:
            for jj in range(nj // 2):
                ps = psum.tile([P, 2, S], _FP32, tag="ps2", bufs=3)
                for half in range(2):
                    j = 2 * jj + half
                    for k in range(KO):
                        nc.tensor.matmul(
                            ps[:, half, :],
                            wh_sb[:, k, j * P:(j + 1) * P],
                            hid_sb[:, k, :],
                            start=(k == 0),
                            stop=(k == KO - 1),
                        )
                for half in range(2):
                    j = 2 * jj + half
                    t = v0 // P + j
                    evict(out_sb[:, j, :], ps[:, half, :], bh_sb[:, t:t + 1], t)
        else:
            for j in range(nj):
                ps = psum.tile([P, S], _FP32, tag="ps")
                for k in range(KO):
                    nc.tensor.matmul(
                        ps[:],
                        wh_sb[:, k, j * P:(j + 1) * P],
                        hid_sb[:, k, :],
                        start=(k == 0),
                        stop=(k == KO - 1),
                    )
                t = v0 // P + j
                evict(out_sb[:, j, :], ps[:], bh_sb[:, t:t + 1], t)
        nc.sync.dma_start(outr[:, v0 // P:v0 // P + nj, :], out_sb[:, :nj, :])
        v0 += vc


_NC_CACHE = {}


def _build_nc(dt_w, n_reps=1):
    key = (str(dt_w), n_reps, BENCH_MODE, EVICT, PAIR_PSUM)
    if key in _NC_CACHE:
        return _NC_CACHE[key]
    nc = bacc.Bacc("TRN2", target_bir_lowering=False, debug=False,
                   num_devices=N_CORES)
    hT = nc.dram_tensor("hT", [D, S], _DT_S1, kind="ExternalInput").ap()
    Wb = nc.dram_tensor("Wb", [D, D], _DT_S1, kind="ExternalInput").ap()
    bbT = nc.dram_tensor("bbT", [P, KO], _FP32, kind="ExternalInput").ap()
    Wh = nc.dram_tensor("Wh", [D, V], dt_w, kind="ExternalInput").ap()
    bhT = nc.dram_tensor("bhT", [P, V // P], _FP32, kind="ExternalInput").ap()
    outT = nc.dram_tensor("outT", [V, S], _FP32, kind="ExternalOutput").ap()
    with tile.TileContext(nc) as tc:
        if n_reps == 1:
            _kernel_body(tc, hT, Wb, bbT, Wh, bhT, outT, dt_w)
        else:
            # Bench-only: repeat the whole computation on-device so the
            # per-iteration time can be separated from dispatch overhead.
            with tc.For_i(0, n_reps, 1):
                _kernel_body(tc, hT, Wb, bbT, Wh, bhT, outT, dt_w)
    nc.compile()
    _NC_CACHE[key] = nc
    return nc


def _make_in_maps(hidden_states, domain_ids, W_base, b_base, W_heads, b_heads,
                  dt_w):
    hidden_states = np.asarray(hidden_states, dtype=np.float32)
    domain_ids = np.asarray(domain_ids)
    W_base = np.ascontiguousarray(np.asarray(W_base, dtype=np.float32))
    b_base = np.asarray(b_base, dtype=np.float32)
    W_heads = np.asarray(W_heads, dtype=np.float32)
    b_heads = np.asarray(b_heads, dtype=np.float32)

    nd = W_heads.shape[0] - 1
    ids = domain_ids.astype(np.int64)
    idx = np.where((ids >= 0) & (ids < nd), ids, nd)

    import ml_dtypes
    bf16 = ml_dtypes.bfloat16
    bbT = np.ascontiguousarray(b_base.reshape(KO, P).T)
    np_w = bf16 if dt_w == _BF16 else np.float32
    Wb_s1 = np.ascontiguousarray(W_base.astype(bf16))

    wh_cache, bh_cache = {}, {}
    in_maps = []
    for b in range(B):
        i = int(idx[b])
        if i not in wh_cache:
            wh_cache[i] = np.ascontiguousarray(
                W_heads[i].astype(np_w, copy=False))
            bh_cache[i] = np.ascontiguousarray(
                b_heads[i].reshape(V // P, P).T)
        in_maps.append({
            "hT": np.ascontiguousarray(hidden_states[b].T.astype(bf16)),
            "Wb": Wb_s1,
            "bbT": bbT,
            "Wh": wh_cache[i],
            "bhT": bh_cache[i],
        })
    return in_maps


def _gather_out(results):
    out = np.empty((B, S, V), dtype=np.float32)
    for b in range(B):
        out[b] = results[b]["outT"].T
    return out


def run_raw(trace=False, **inputs):
    """Run on hardware; returns (out [B,S,V] fp32, BassKernelResults)."""
    dt_w = _BF16 if W_DTYPE == "bfloat16" else _FP32R
    nc = _build_nc(dt_w)
    in_maps = _make_in_maps(
        inputs["hidden_states"], inputs["domain_ids"], inputs["W_base"],
        inputs["b_base"], inputs["W_heads"], inputs["b_heads"], dt_w)
    res = run_bass_kernel_spmd(nc, in_maps, core_ids=list(range(N_CORES)),
                               trace=trace)
    return _gather_out(res.results), res


def kernel(**inputs) -> np.ndarray:
    out, _ = run_raw(trace=False, **inputs)
    return out


# ---------------------------------------------------------------------------
# Dev-only helpers below (not used by kernel()).
# ---------------------------------------------------------------------------

def predict_ns():
    """Cost-model (TimelineSim) predicted single-core duration in ns."""
    from concourse.timeline_sim import TimelineSim
    dt_w = _BF16 if W_DTYPE == "bfloat16" else _FP32R
    nc = _build_nc(dt_w)
    tl = TimelineSim(nc, trace=False)
    return tl.simulate()


def _make_runner(nc, in_maps):
    """Build a jitted single-dispatch runner over device-resident inputs.
    Returns (run_once, to_out_maps)."""
    import jax
    from jax.sharding import Mesh, PartitionSpec, NamedSharding
    from jax.experimental.shard_map import shard_map
    from concourse import bass2jax
    from concourse import mybir as _mybir

    bass2jax.install_neuronx_cc_hook()
    partition_name = (nc.partition_id_tensor.name
                      if nc.partition_id_tensor else None)
    in_names, out_names, out_avals, zero_outs = [], [], [], []
    for alloc in nc.m.functions[0].allocations:
        if not isinstance(alloc, _mybir.MemoryLocationSet):
            continue
        name = alloc.memorylocations[0].name
        if alloc.kind == "ExternalInput":
            if name != partition_name:
                in_names.append(name)
        elif alloc.kind == "ExternalOutput":
            out_names.append(name)
            shape = tuple(alloc.tensor_shape)
            dtype = _mybir.dt.np(alloc.dtype)
            out_avals.append(jax.core.ShapedArray(shape, dtype))
            zero_outs.append(np.zeros(shape, dtype))
    n_params = len(in_names)
    n_outs = len(out_avals)
    all_names = in_names + out_names
    if partition_name is not None:
        all_names = all_names + [partition_name]

    def _body(*args):
        operands = list(args)
        if partition_name is not None:
            operands.append(bass2jax.partition_id_tensor())
        return tuple(bass2jax._bass_exec_p.bind(
            *operands,
            out_avals=tuple(out_avals),
            in_names=tuple(all_names),
            out_names=tuple(out_names),
            lowering_input_output_aliases=(),
            sim_require_finite=True,
            sim_require_nnan=True,
            nc=nc,
        ))

    devices = jax.devices()[:N_CORES]
    mesh = Mesh(np.asarray(devices), ("core",))
    spec = PartitionSpec("core")
    f = jax.jit(
        shard_map(_body, mesh=mesh, in_specs=(spec,) * (n_params + n_outs),
                  out_specs=(spec,) * n_outs, check_rep=False),
        keep_unused=True)

    sharding = NamedSharding(mesh, spec)
    concat_in = [
        jax.device_put(
            np.concatenate([np.asarray(in_maps[c][nm]) for c in range(N_CORES)],
                           axis=0), sharding)
        for nm in in_names
    ]
    jax.block_until_ready(concat_in)

    z = [jax.device_put(
        np.zeros((N_CORES * zz.shape[0], *zz.shape[1:]), zz.dtype), sharding)
        for zz in zero_outs]
    jax.block_until_ready(z)

    def run_once():
        import time
        t0 = time.perf_counter()
        outs = f(*concat_in, *z)
        jax.block_until_ready(outs)
        return time.perf_counter() - t0, outs

    def to_out_maps(outs):
        return [
            {nm: np.asarray(outs[i]).reshape(N_CORES, *out_avals[i].shape)[c]
             for i, nm in enumerate(out_names)}
            for c in range(N_CORES)
        ]

    return run_once, to_out_maps


def bench(n_iters=16, **inputs):
    """Measure per-kernel HW time: build a NEFF that repeats the body
    n_iters times in a hardware loop and difference against the 1-rep NEFF.
    Returns (out, per_iter_ns, first_total_ns)."""
    import time
    dt_w = _BF16 if W_DTYPE == "bfloat16" else _FP32R
    in_maps = _make_in_maps(
        inputs["hidden_states"], inputs["domain_ids"], inputs["W_base"],
        inputs["b_base"], inputs["W_heads"], inputs["b_heads"], dt_w)

    nc1 = _build_nc(dt_w, 1)
    run1, to_out_maps = _make_runner(nc1, in_maps)
    t0 = time.perf_counter()
    _, outs = run1()
    first_total = time.perf_counter() - t0

    ncn = _build_nc(dt_w, n_iters)
    runn, _ = _make_runner(ncn, in_maps)
    runn()  # warm

    # Interleaved sweeps: cross-run drift on the shared device is large, so
    # take the min per-iteration estimate across several paired measurements.
    per_iter = float("inf")
    for _ in range(4):
        t1 = min(run1()[0] for _ in range(3))
        tn = min(runn()[0] for _ in range(3))
        per_iter = min(per_iter, (tn - t1) / (n_iters - 1))
    _, outs = run1()

    return _gather_out(to_out_maps(outs)), per_iter * 1e9, first_total * 1e9
